# revision 21
# baseline (speedup 1.0000x reference)
"""Trainium2 kernel for nn_BpsMlp: KNN min-distance (B=64,N=1024 queries vs
M=4096 basis points) feeding a 4-layer MLP, data-parallel over batch across
8 NeuronCores.

Per core (8 batches = 8192 query rows):
  - distance phase: d2[q,m] accumulated exactly in fp32 PSUM via K=16
    augmented bf16 hi/lo matmuls (catastrophic-cancellation-free), four
    matmuls packed concurrently into the PE via tile_position row-groups.
  - drain: per q-tile the 4096 d2 values sit in 4 PSUM chunks of 1024.
    ScalarE (the cheaper PSUM-exit engine) casts chunks 0-2 to fp16 SBUF;
    VectorE pair-mins chunk 3 against cast2 (retiring 2 elems/cycle), folds
    the remaining fp16 with 2x-mode TTs, and finishes with a fused
    min-accumulate into the per-query x column. V-queue ordered so its
    first op per tile waits only on the first two casts.
  - x = sqrt(max(d2min, 1e-12)) with one Newton refinement step.
  - MLP in fp16 (weights streamed to SBUF during the distance phase),
    h^T layout [hid-tile 128, batch 8], relu+bias on VectorE.
"""

import sys

sys.path.insert(0, "/opt/trn_rl_repo")

import numpy as np
import ml_dtypes

import concourse.bass as bass
import concourse.mybir as mybir
import concourse.tile as tile
from concourse.bass import ds, ts
from concourse.bass_utils import run_bass_kernel_spmd

BF16 = ml_dtypes.bfloat16
DT = mybir.dt
AF = mybir.ActivationFunctionType
OP = mybir.AluOpType

B, N, M = 64, 1024, 4096
HID, OUT = 2048, 512
NCORES = 8
BPC = B // NCORES            # batches per core
R = BPC * N                  # query rows per core (8192)
QT = R // 128                # q-tiles per core (64)
KAUG = 16                    # augmented contraction dim
MT_H = HID // 128            # hid tiles (16)
KT1 = N // 128               # L1 k-tiles (8)
KT2 = HID // 128             # L2/L3/L4 k-tiles (16)
MT_O = OUT // 128            # out tiles (4)

_cache = {}


def _split_hi_lo(v):
    vh = v.astype(BF16).astype(np.float32)
    vl = (v - vh).astype(BF16).astype(np.float32)
    return vh, vl


def _build_program():
    nc = bass.Bass()

    posT = nc.declare_dram_parameter("posT_aug", [128, R], DT.bfloat16, isOutput=False)
    basisA = nc.declare_dram_parameter("basis_aug", [128, M], DT.bfloat16, isOutput=False)
    w0 = nc.declare_dram_parameter("w0", [128, KT1 * HID], DT.float16, isOutput=False)
    w1 = nc.declare_dram_parameter("w1", [128, KT2 * HID], DT.float16, isOutput=False)
    w2 = nc.declare_dram_parameter("w2", [128, KT2 * HID], DT.float16, isOutput=False)
    w3 = nc.declare_dram_parameter("w3", [128, KT2 * OUT], DT.float16, isOutput=False)
    b0d = nc.declare_dram_parameter("b0t", [128, MT_H], DT.float32, isOutput=False)
    b1d = nc.declare_dram_parameter("b1t", [128, MT_H], DT.float32, isOutput=False)
    b2d = nc.declare_dram_parameter("b2t", [128, MT_H], DT.float32, isOutput=False)
    b3d = nc.declare_dram_parameter("b3t", [128, MT_O], DT.float32, isOutput=False)
    outT = nc.declare_dram_parameter("outT", [MT_O, 128, BPC], DT.float32, isOutput=True)

    with tile.TileContext(nc) as tc:
        with (
            tc.tile_pool(name="const", bufs=1) as const,
            tc.tile_pool(name="dpsum", bufs=4, space="PSUM") as dpsum,
            tc.tile_pool(name="castp", bufs=2) as castp,
            tc.tile_pool(name="c2cp", bufs=2) as c2cp,
            tc.tile_pool(name="foldp", bufs=1) as foldp,
            tc.tile_pool(name="hpool", bufs=2) as hpool,
            tc.tile_pool(name="junkp", bufs=1) as junkp,
            tc.tile_pool(name="posc", bufs=2) as posc,
        ):
            basis_sb = const.tile([128, M], DT.bfloat16)
            pos_tiles = {}

            def issue_chunk(c, engine=None):
                # c indexes half-chunks of 512 query-columns (4 q-tiles)
                pc_ = posc.tile([128, 512], DT.bfloat16, tag="posc")
                e0 = engine if engine else nc.sync
                e0.dma_start(pc_[:], posT[:, ds(c * 512, 512)])
                pos_tiles[c] = pc_

            # ramp: spread the first DMAs across engine queues so their
            # ~0.6us issue costs don't serialize on the sync queue.
            issue_chunk(0, engine=nc.scalar)
            issue_chunk(1, engine=nc.scalar)
            # first basis quarter split in two so q-tile 0's first matmuls
            # wait on a 128KB transfer, not 256KB; weight DMAs wait until
            # the second chunk boundary to keep the queues clear.
            nc.gpsimd.dma_start(basis_sb[:, 0:512], basisA[:, 0:512])
            nc.sync.dma_start(basis_sb[:, 512:1024], basisA[:, 512:1024])
            for j in range(1, 4):
                nc.sync.dma_start(basis_sb[:, ts(j, 1024)], basisA[:, ts(j, 1024)])

            w0_sb = const.tile([128, KT1 * HID], DT.float16)
            w1_sb = const.tile([128, KT2 * HID], DT.float16)
            w2_sb = const.tile([128, KT2 * HID], DT.float16)
            w3_sb = const.tile([128, KT2 * OUT], DT.float16)
            b0_sb = const.tile([128, MT_H], DT.float32)
            b1_sb = const.tile([128, MT_H], DT.float32)
            b2_sb = const.tile([128, MT_H], DT.float32)
            b3_sb = const.tile([128, MT_O], DT.float32)

            x1 = const.tile([128, QT], DT.float32)
            junk = junkp.tile([128, 512], DT.float16)

            # MLP weight DMAs are spread across the distance phase so the
            # pos-chunk prefetches never sit behind a deep weight backlog.
            wdmas = []
            for j in range(KT1):
                wdmas.append((w0_sb[:, ts(j, HID)], w0[:, ts(j, HID)]))
            for j in range(KT2):
                wdmas.append((w1_sb[:, ts(j, HID)], w1[:, ts(j, HID)]))
                wdmas.append((w2_sb[:, ts(j, HID)], w2[:, ts(j, HID)]))
                wdmas.append((w3_sb[:, ts(j, OUT)], w3[:, ts(j, OUT)]))
            wdmas.append((b0_sb[:], b0d[:]))
            wdmas.append((b1_sb[:], b1d[:]))
            wdmas.append((b2_sb[:], b2d[:]))
            wdmas.append((b3_sb[:], b3d[:]))
            wd_i = 0

            # ---- distance phase ----
            for t in range(QT):
                if t % 4 == 0:
                    c = t // 4
                    if c + 2 < QT // 4:
                        issue_chunk(c + 2)
                    n_issue = (len(wdmas) * c) // (QT // 4 - 1) - wd_i
                    for _ in range(n_issue):
                        dst, src = wdmas[wd_i]
                        nc.sync.dma_start(dst, src)
                        wd_i += 1
                pos_chunk = pos_tiles[t // 4]
                # S-heavy drain: ScalarE casts chunks 0-2 to fp16 (its PSUM
                # exits are cheaper than VectorE's); VectorE pair-mins chunk 3
                # against cast2, then folds [cast0|cast1] + s3 with 2x fp16
                # TTs and a final fused min-accum into the x column.
                # PE fills the pair chunk (m-chunk 3) first; VectorE's pair
                # then only waits on ScalarE's FIRST cast of the tile, and the
                # independent u-fold consumes casts 1,2 as they land. Shallow
                # tree (pair || u) -> v -> w -> accum keeps V latency low.
                cb0 = castp.tile([128, 1024], DT.float16, tag="c01")
                cb1 = castp.tile([128, 1024], DT.float16, tag="c01")
                c2cast = c2cp.tile([128, 1024], DT.float16, tag="c2c")
                pt3 = None
                for ci in (3, 0, 1, 2):
                    pt = dpsum.tile([128, 1024], DT.float32, tag="d")
                    for h in range(2):
                        g = 2 * (ci % 2) + h
                        nc.tensor.matmul(
                            pt[:, ts(h, 512)],
                            pos_chunk[32 * g : 32 * g + KAUG, ts(t % 4, 128)],
                            basis_sb[32 * g : 32 * g + KAUG, ds(1024 * ci + 512 * h, 512)],
                            tile_position=(32 * g, 0),
                        )
                    if ci == 3:
                        pt3 = pt
                    elif ci == 0:
                        nc.scalar.copy(cb0[:], pt[:])
                        s3 = foldp.tile([128, 1024], DT.float16, tag="s3")
                        nc.vector.tensor_tensor(s3[:], pt3[:], cb0[:], op=OP.min)
                    elif ci == 1:
                        nc.scalar.copy(cb1[:], pt[:])
                    else:
                        nc.scalar.copy(c2cast[:], pt[:])
                        u = foldp.tile([128, 1024], DT.float16, tag="u")
                        nc.vector.tensor_tensor(u[:], cb1[:], c2cast[:], op=OP.min)
                v = foldp.tile([128, 1024], DT.float16, tag="v")
                nc.vector.tensor_tensor(v[:], u[:], s3[:], op=OP.min)
                w = foldp.tile([128, 512], DT.float16, tag="w")
                nc.vector.tensor_tensor(w[:], v[:, 0:512], v[:, 512:1024], op=OP.min)
                nc.vector.tensor_scalar(
                    junk[:], w[:], 1.0, None,
                    op0=OP.mult, op1=OP.min, accum_out=x1[:, t : t + 1],
                )

            # ---- x = sqrt(max(d2,1e-12)), one Newton step ----
            xc = const.tile([128, QT], DT.float32)
            nc.vector.tensor_scalar_max(xc[:], x1[:], 1e-12)
            y0 = const.tile([128, QT], DT.float32)
            nc.scalar.activation(y0[:], xc[:], AF.Sqrt)
            ry = const.tile([128, QT], DT.float32)
            nc.vector.reciprocal(ry[:], y0[:])
            t1 = const.tile([128, QT], DT.float32)
            nc.vector.tensor_mul(t1[:], xc[:], ry[:])
            nc.vector.tensor_add(x1[:], y0[:], t1[:])
            xbf = const.tile([128, QT], DT.float16)
            nc.vector.tensor_scalar_mul(xbf[:], x1[:], 0.5)

            # ---- MLP (h^T layout: [hid-tile 128, batch 8]) ----
            xg = xbf[:].rearrange("p (b t) -> p t b", t=KT1)
            zero_t = const.tile([128, BPC], DT.float16)
            nc.vector.memset(zero_t[:], 0.0)

            def layer(in_view, w_sb, b_sb, n_kt, n_mt, act_relu, out_dtype):
                # One small psum tile per mt-group: consecutive groups
                # ping-pong the two pool slots, so the relu's PSUM read never
                # serializes against the next group's matmuls.
                hout = hpool.tile([128, n_mt * BPC], out_dtype, tag="h" + str(n_mt))
                for mt in range(n_mt):
                    # MLP psum tiles share the distance pool's ring (first
                    # BPC columns of a [128, 1024] slot) -- saves 2 banks so
                    # the distance phase can quad-buffer.
                    ptf = dpsum.tile([128, 1024], DT.float32, tag="d")
                    pt = ptf[:, 0:BPC]
                    for kt in range(n_kt):
                        nc.tensor.matmul(
                            pt,
                            w_sb[:, ds(kt * n_mt * 128 + mt * 128, 128)],
                            in_view[:, kt, :],
                            start=(kt == 0),
                            stop=(kt == n_kt - 1),
                        )
                    if act_relu:
                        # relu(psum + bias) on VectorE (idle during MLP)
                        nc.vector.scalar_tensor_tensor(
                            hout[:, ds(mt * BPC, BPC)],
                            pt,
                            b_sb[:, mt : mt + 1],
                            zero_t[:],
                            op0=OP.add,
                            op1=OP.max,
                        )
                    else:
                        nc.scalar.activation(
                            hout[:, ds(mt * BPC, BPC)],
                            pt,
                            AF.Identity,
                            bias=b_sb[:, mt : mt + 1],
                        )
                return hout

            h1 = layer(xg, w0_sb, b0_sb, KT1, MT_H, True, DT.float16)
            h1v = h1[:].rearrange("p (t b) -> p t b", b=BPC)
            h2 = layer(h1v, w1_sb, b1_sb, KT2, MT_H, True, DT.float16)
            h2v = h2[:].rearrange("p (t b) -> p t b", b=BPC)
            h3 = layer(h2v, w2_sb, b2_sb, KT2, MT_H, True, DT.float16)
            h3v = h3[:].rearrange("p (t b) -> p t b", b=BPC)
            h4 = layer(h3v, w3_sb, b3_sb, KT2, MT_O, False, DT.float32)

            for mt in range(MT_O):
                nc.sync.dma_start(outT[mt], h4[:, ds(mt * BPC, BPC)])

    _split_multi_waits(nc)
    return nc


def _split_multi_waits(nc, max_waits=1):
    """neuronx-cc walrus rejects instructions with >1 sync wait; hoist extras
    onto nofuse NOPs just before, on the same engine."""
    ctr = 0
    for f in nc.m.functions:
        for bb in f.blocks:
            new_insts = []
            for ins in bb.instructions:
                si = getattr(ins, "sync_info", None)
                if si is not None and si.on_wait and len(si.on_wait) > max_waits:
                    waits = list(si.on_wait)
                    extra, keep = waits[:-max_waits], waits[-max_waits:]
                    for i in range(0, len(extra), max_waits):
                        ctr += 1
                        new_insts.append(
                            mybir.InstNoOp(
                                name=f"waitsplit-{ctr}",
                                engine=ins.engine,
                                sync_info=mybir.SyncInfo(
                                    on_wait=extra[i : i + max_waits], on_update=[]
                                ),
                                bass_nofuse=True,
                            )
                        )
                    si.on_wait = keep
                new_insts.append(ins)
            bb.instructions[:] = new_insts


def _prep_inputs(pos, basis, W0, b0, W1, b1, W2, b2, W3, b3):
    pos = np.asarray(pos, dtype=np.float32)
    basis = np.asarray(basis, dtype=np.float32)

    bh, bl = _split_hi_lo(basis)  # [M,3]
    q2 = (basis * basis).sum(-1)
    q2h, q2l = _split_hi_lo(q2)
    ones_m = np.ones(M, np.float32)
    basis_aug = np.zeros((16, M), np.float32)
    basis_aug[0:3] = bh.T
    basis_aug[3:6] = bh.T
    basis_aug[6:9] = bl.T
    basis_aug[9:12] = bl.T
    basis_aug[12] = ones_m
    basis_aug[13] = ones_m
    basis_aug[14] = q2h
    basis_aug[15] = q2l
    # replicate into the 4 PE row-groups (partitions 32g..32g+15)
    basis_rep = np.zeros((128, M), np.float32)
    for g in range(4):
        basis_rep[32 * g : 32 * g + 16] = basis_aug
    basis_rep = basis_rep.astype(BF16)

    def pos_aug_for_core(c):
        p = pos[c * BPC : (c + 1) * BPC].reshape(R, 3)
        a = -2.0 * p
        ah, al = _split_hi_lo(a)
        p2 = (p * p).sum(-1)
        p2h, p2l = _split_hi_lo(p2)
        ones_r = np.ones(R, np.float32)
        pa = np.zeros((16, R), np.float32)
        pa[0:3] = ah.T
        pa[3:6] = al.T
        pa[6:9] = ah.T
        pa[9:12] = al.T
        pa[12] = p2h
        pa[13] = p2l
        pa[14] = ones_r
        pa[15] = ones_r
        pa_rep = np.zeros((128, R), np.float32)
        for g in range(4):
            pa_rep[32 * g : 32 * g + 16] = pa
        return pa_rep.astype(BF16)

    def pack_w(W, n_kt, n_out):
        return (
            np.asarray(W, np.float32)
            .reshape(n_kt, 128, n_out)
            .transpose(1, 0, 2)
            .reshape(128, n_kt * n_out)
            .astype(np.float16)
        )

    common = {
        "basis_aug": basis_rep,
        "w0": pack_w(W0, KT1, HID),
        "w1": pack_w(W1, KT2, HID),
        "w2": pack_w(W2, KT2, HID),
        "w3": pack_w(W3, KT2, OUT),
        "b0t": np.asarray(b0, np.float32).reshape(MT_H, 128).T.copy(),
        "b1t": np.asarray(b1, np.float32).reshape(MT_H, 128).T.copy(),
        "b2t": np.asarray(b2, np.float32).reshape(MT_H, 128).T.copy(),
        "b3t": np.asarray(b3, np.float32).reshape(MT_O, 128).T.copy(),
    }
    in_maps = []
    for c in range(NCORES):
        m = dict(common)
        m["posT_aug"] = pos_aug_for_core(c)
        in_maps.append(m)
    return in_maps


def kernel(pos, basis, W0, b0, W1, b1, W2, b2, W3, b3, _trace=False):
    if "nc" not in _cache:
        _cache["nc"] = _build_program()
    nc = _cache["nc"]
    in_maps = _prep_inputs(pos, basis, W0, b0, W1, b1, W2, b2, W3, b3)
    res = run_bass_kernel_spmd(nc, in_maps, list(range(NCORES)), trace=_trace)
    _cache["last_result"] = res
    out = np.empty((B, OUT), np.float32)
    for c in range(NCORES):
        o = np.asarray(res.results[c]["outT"])  # [MT_O, 128, BPC]
        out[c * BPC : (c + 1) * BPC] = o.transpose(2, 0, 1).reshape(BPC, OUT)
    return out


# revision 22
# speedup vs baseline: 1.0071x; 1.0071x over previous
"""Trainium2 kernel for nn_BpsMlp: KNN min-distance (B=64,N=1024 queries vs
M=4096 basis points) feeding a 4-layer MLP, data-parallel over batch across
8 NeuronCores.

Per core (8 batches = 8192 query rows):
  - distance phase: d2[q,m] accumulated exactly in fp32 PSUM via K=16
    augmented bf16 hi/lo matmuls (catastrophic-cancellation-free), four
    matmuls packed concurrently into the PE via tile_position row-groups.
  - drain: per q-tile the 4096 d2 values sit in 4 PSUM chunks of 1024.
    ScalarE (the cheaper PSUM-exit engine) casts chunks 0-2 to fp16 SBUF;
    VectorE pair-mins chunk 3 against cast2 (retiring 2 elems/cycle), folds
    the remaining fp16 with 2x-mode TTs, and finishes with a fused
    min-accumulate into the per-query x column. V-queue ordered so its
    first op per tile waits only on the first two casts.
  - x = sqrt(max(d2min, 1e-12)) with one Newton refinement step.
  - MLP in fp16 (weights streamed to SBUF during the distance phase),
    h^T layout [hid-tile 128, batch 8], relu+bias on VectorE.
"""

import sys

sys.path.insert(0, "/opt/trn_rl_repo")

import numpy as np
import ml_dtypes

import concourse.bass as bass
import concourse.mybir as mybir
import concourse.tile as tile
from concourse.bass import ds, ts
from concourse.bass_utils import run_bass_kernel_spmd

BF16 = ml_dtypes.bfloat16
DT = mybir.dt
AF = mybir.ActivationFunctionType
OP = mybir.AluOpType

B, N, M = 64, 1024, 4096
HID, OUT = 2048, 512
NCORES = 8
BPC = B // NCORES            # batches per core
R = BPC * N                  # query rows per core (8192)
QT = R // 128                # q-tiles per core (64)
KAUG = 16                    # augmented contraction dim
MT_H = HID // 128            # hid tiles (16)
KT1 = N // 128               # L1 k-tiles (8)
KT2 = HID // 128             # L2/L3/L4 k-tiles (16)
MT_O = OUT // 128            # out tiles (4)

_cache = {}


def _split_hi_lo(v):
    vh = v.astype(BF16).astype(np.float32)
    vl = (v - vh).astype(BF16).astype(np.float32)
    return vh, vl


def _build_program():
    nc = bass.Bass()

    posT = nc.declare_dram_parameter("posT_aug", [128, R], DT.bfloat16, isOutput=False)
    basisA = nc.declare_dram_parameter("basis_aug", [128, M], DT.bfloat16, isOutput=False)
    w0 = nc.declare_dram_parameter("w0", [128, KT1 * HID], DT.float16, isOutput=False)
    w1 = nc.declare_dram_parameter("w1", [128, KT2 * HID], DT.float16, isOutput=False)
    w2 = nc.declare_dram_parameter("w2", [128, KT2 * HID], DT.float16, isOutput=False)
    w3 = nc.declare_dram_parameter("w3", [128, KT2 * OUT], DT.float16, isOutput=False)
    b0d = nc.declare_dram_parameter("b0t", [128, MT_H], DT.float32, isOutput=False)
    b1d = nc.declare_dram_parameter("b1t", [128, MT_H], DT.float32, isOutput=False)
    b2d = nc.declare_dram_parameter("b2t", [128, MT_H], DT.float32, isOutput=False)
    b3d = nc.declare_dram_parameter("b3t", [128, MT_O], DT.float32, isOutput=False)
    outT = nc.declare_dram_parameter("outT", [MT_O, 128, BPC], DT.float32, isOutput=True)

    with tile.TileContext(nc) as tc:
        with (
            tc.tile_pool(name="const", bufs=1) as const,
            tc.tile_pool(name="dpsum", bufs=4, space="PSUM") as dpsum,
            tc.tile_pool(name="castp", bufs=2) as castp,
            tc.tile_pool(name="c2cp", bufs=2) as c2cp,
            tc.tile_pool(name="foldp", bufs=1) as foldp,
            tc.tile_pool(name="hpool", bufs=2) as hpool,
            tc.tile_pool(name="junkp", bufs=1) as junkp,
            tc.tile_pool(name="posc", bufs=2) as posc,
        ):
            basis_sb = const.tile([128, M], DT.bfloat16)
            pos_tiles = {}

            def issue_chunk(c, engine=None):
                # c indexes half-chunks of 512 query-columns (4 q-tiles)
                pc_ = posc.tile([128, 512], DT.bfloat16, tag="posc")
                e0 = engine if engine else nc.sync
                e0.dma_start(pc_[:], posT[:, ds(c * 512, 512)])
                pos_tiles[c] = pc_

            # ramp: spread the first DMAs across engine queues so their
            # ~0.6us issue costs don't serialize on the sync queue.
            issue_chunk(0, engine=nc.scalar)
            issue_chunk(1, engine=nc.scalar)
            # first basis quarter split in two so q-tile 0's first matmuls
            # wait on a 128KB transfer, not 256KB; weight DMAs wait until
            # the second chunk boundary to keep the queues clear.
            nc.gpsimd.dma_start(basis_sb[:, 0:512], basisA[:, 0:512])
            nc.sync.dma_start(basis_sb[:, 512:1024], basisA[:, 512:1024])
            for j in range(1, 4):
                nc.sync.dma_start(basis_sb[:, ts(j, 1024)], basisA[:, ts(j, 1024)])

            w0_sb = const.tile([128, KT1 * HID], DT.float16)
            w1_sb = const.tile([128, KT2 * HID], DT.float16)
            w2_sb = const.tile([128, KT2 * HID], DT.float16)
            w3_sb = const.tile([128, KT2 * OUT], DT.float16)
            b0_sb = const.tile([128, MT_H], DT.float32)
            b1_sb = const.tile([128, MT_H], DT.float32)
            b2_sb = const.tile([128, MT_H], DT.float32)
            b3_sb = const.tile([128, MT_O], DT.float32)

            x1 = const.tile([128, QT], DT.float32)
            junk = junkp.tile([128, 512], DT.float16)

            # MLP weight DMAs are spread across the distance phase so the
            # pos-chunk prefetches never sit behind a deep weight backlog.
            wdmas = []
            for j in range(KT1):
                wdmas.append((w0_sb[:, ts(j, HID)], w0[:, ts(j, HID)]))
            for j in range(KT2):
                wdmas.append((w1_sb[:, ts(j, HID)], w1[:, ts(j, HID)]))
                wdmas.append((w2_sb[:, ts(j, HID)], w2[:, ts(j, HID)]))
                wdmas.append((w3_sb[:, ts(j, OUT)], w3[:, ts(j, OUT)]))
            wdmas.append((b0_sb[:], b0d[:]))
            wdmas.append((b1_sb[:], b1d[:]))
            wdmas.append((b2_sb[:], b2d[:]))
            wdmas.append((b3_sb[:], b3d[:]))
            wd_i = 0

            # ---- distance phase ----
            for t in range(QT):
                if t % 4 == 0:
                    c = t // 4
                    if c + 2 < QT // 4:
                        issue_chunk(c + 2)
                    n_issue = (len(wdmas) * c) // (QT // 4 - 1) - wd_i
                    for _ in range(n_issue):
                        dst, src = wdmas[wd_i]
                        nc.sync.dma_start(dst, src)
                        wd_i += 1
                pos_chunk = pos_tiles[t // 4]
                # S-heavy drain: ScalarE casts chunks 0-2 to fp16 (its PSUM
                # exits are cheaper than VectorE's); VectorE pair-mins chunk 3
                # against cast2, then folds [cast0|cast1] + s3 with 2x fp16
                # TTs and a final fused min-accum into the x column.
                cb0 = castp.tile([128, 1024], DT.float16, tag="c01")
                cb1 = castp.tile([128, 1024], DT.float16, tag="c01")
                c2cast = c2cp.tile([128, 1024], DT.float16, tag="c2c")
                for ci in range(4):
                    pt = dpsum.tile([128, 1024], DT.float32, tag="d")
                    for h in range(2):
                        g = 2 * (ci % 2) + h
                        nc.tensor.matmul(
                            pt[:, ts(h, 512)],
                            pos_chunk[32 * g : 32 * g + KAUG, ts(t % 4, 128)],
                            basis_sb[32 * g : 32 * g + KAUG, ds(1024 * ci + 512 * h, 512)],
                            tile_position=(32 * g, 0),
                        )
                    if ci < 2:
                        nc.scalar.copy((cb0 if ci == 0 else cb1)[:], pt[:])
                        if ci == 1:
                            # issue the cast0/cast1 fold before the pair so
                            # VectorE's first op of this tile only waits on
                            # the first two casts, not the third.
                            u = foldp.tile([128, 1024], DT.float16, tag="u")
                            nc.vector.tensor_tensor(
                                u[:], cb0[:], cb1[:], op=OP.min
                            )
                    elif ci == 2:
                        nc.scalar.copy(c2cast[:], pt[:])
                    else:
                        s3 = foldp.tile([128, 1024], DT.float16, tag="s3")
                        nc.vector.tensor_tensor(s3[:], pt[:], c2cast[:], op=OP.min)
                v = foldp.tile([128, 1024], DT.float16, tag="v")
                nc.vector.tensor_tensor(v[:], u[:], s3[:], op=OP.min)
                w = foldp.tile([128, 512], DT.float16, tag="w")
                nc.vector.tensor_tensor(w[:], v[:, 0:512], v[:, 512:1024], op=OP.min)
                nc.vector.tensor_scalar(
                    junk[:], w[:], 1.0, None,
                    op0=OP.mult, op1=OP.min, accum_out=x1[:, t : t + 1],
                )

            # ---- x = sqrt(max(d2,1e-12)), one Newton step ----
            xc = const.tile([128, QT], DT.float32)
            nc.vector.tensor_scalar_max(xc[:], x1[:], 1e-12)
            y0 = const.tile([128, QT], DT.float32)
            nc.scalar.activation(y0[:], xc[:], AF.Sqrt)
            ry = const.tile([128, QT], DT.float32)
            nc.vector.reciprocal(ry[:], y0[:])
            t1 = const.tile([128, QT], DT.float32)
            nc.vector.tensor_mul(t1[:], xc[:], ry[:])
            nc.vector.tensor_add(x1[:], y0[:], t1[:])
            xbf = const.tile([128, QT], DT.float16)
            nc.vector.tensor_scalar_mul(xbf[:], x1[:], 0.5)

            # ---- MLP (h^T layout: [hid-tile 128, batch 8]) ----
            xg = xbf[:].rearrange("p (b t) -> p t b", t=KT1)
            zero_t = const.tile([128, BPC], DT.float16)
            nc.vector.memset(zero_t[:], 0.0)

            def layer(in_view, w_sb, b_sb, n_kt, n_mt, act_relu, out_dtype):
                # One small psum tile per mt-group: consecutive groups
                # ping-pong the two pool slots, so the relu's PSUM read never
                # serializes against the next group's matmuls.
                hout = hpool.tile([128, n_mt * BPC], out_dtype, tag="h" + str(n_mt))
                for mt in range(n_mt):
                    # MLP psum tiles share the distance pool's ring (first
                    # BPC columns of a [128, 1024] slot) -- saves 2 banks so
                    # the distance phase can quad-buffer.
                    ptf = dpsum.tile([128, 1024], DT.float32, tag="d")
                    pt = ptf[:, 0:BPC]
                    for kt in range(n_kt):
                        nc.tensor.matmul(
                            pt,
                            w_sb[:, ds(kt * n_mt * 128 + mt * 128, 128)],
                            in_view[:, kt, :],
                            start=(kt == 0),
                            stop=(kt == n_kt - 1),
                        )
                    if act_relu:
                        # relu(psum + bias) on VectorE (idle during MLP)
                        nc.vector.scalar_tensor_tensor(
                            hout[:, ds(mt * BPC, BPC)],
                            pt,
                            b_sb[:, mt : mt + 1],
                            zero_t[:],
                            op0=OP.add,
                            op1=OP.max,
                        )
                    else:
                        nc.scalar.activation(
                            hout[:, ds(mt * BPC, BPC)],
                            pt,
                            AF.Identity,
                            bias=b_sb[:, mt : mt + 1],
                        )
                return hout

            h1 = layer(xg, w0_sb, b0_sb, KT1, MT_H, True, DT.float16)
            h1v = h1[:].rearrange("p (t b) -> p t b", b=BPC)
            h2 = layer(h1v, w1_sb, b1_sb, KT2, MT_H, True, DT.float16)
            h2v = h2[:].rearrange("p (t b) -> p t b", b=BPC)
            h3 = layer(h2v, w2_sb, b2_sb, KT2, MT_H, True, DT.float16)
            h3v = h3[:].rearrange("p (t b) -> p t b", b=BPC)
            h4 = layer(h3v, w3_sb, b3_sb, KT2, MT_O, False, DT.float32)

            for mt in range(MT_O):
                nc.sync.dma_start(outT[mt], h4[:, ds(mt * BPC, BPC)])

    _split_multi_waits(nc)
    return nc


def _split_multi_waits(nc, max_waits=1):
    """neuronx-cc walrus rejects instructions with >1 sync wait; hoist extras
    onto nofuse NOPs just before, on the same engine."""
    ctr = 0
    for f in nc.m.functions:
        for bb in f.blocks:
            new_insts = []
            for ins in bb.instructions:
                si = getattr(ins, "sync_info", None)
                if si is not None and si.on_wait and len(si.on_wait) > max_waits:
                    waits = list(si.on_wait)
                    extra, keep = waits[:-max_waits], waits[-max_waits:]
                    for i in range(0, len(extra), max_waits):
                        ctr += 1
                        new_insts.append(
                            mybir.InstNoOp(
                                name=f"waitsplit-{ctr}",
                                engine=ins.engine,
                                sync_info=mybir.SyncInfo(
                                    on_wait=extra[i : i + max_waits], on_update=[]
                                ),
                                bass_nofuse=True,
                            )
                        )
                    si.on_wait = keep
                new_insts.append(ins)
            bb.instructions[:] = new_insts


def _prep_inputs(pos, basis, W0, b0, W1, b1, W2, b2, W3, b3):
    pos = np.asarray(pos, dtype=np.float32)
    basis = np.asarray(basis, dtype=np.float32)

    bh, bl = _split_hi_lo(basis)  # [M,3]
    q2 = (basis * basis).sum(-1)
    q2h, q2l = _split_hi_lo(q2)
    ones_m = np.ones(M, np.float32)
    basis_aug = np.zeros((16, M), np.float32)
    basis_aug[0:3] = bh.T
    basis_aug[3:6] = bh.T
    basis_aug[6:9] = bl.T
    basis_aug[9:12] = bl.T
    basis_aug[12] = ones_m
    basis_aug[13] = ones_m
    basis_aug[14] = q2h
    basis_aug[15] = q2l
    # replicate into the 4 PE row-groups (partitions 32g..32g+15)
    basis_rep = np.zeros((128, M), np.float32)
    for g in range(4):
        basis_rep[32 * g : 32 * g + 16] = basis_aug
    basis_rep = basis_rep.astype(BF16)

    def pos_aug_for_core(c):
        p = pos[c * BPC : (c + 1) * BPC].reshape(R, 3)
        a = -2.0 * p
        ah, al = _split_hi_lo(a)
        p2 = (p * p).sum(-1)
        p2h, p2l = _split_hi_lo(p2)
        ones_r = np.ones(R, np.float32)
        pa = np.zeros((16, R), np.float32)
        pa[0:3] = ah.T
        pa[3:6] = al.T
        pa[6:9] = ah.T
        pa[9:12] = al.T
        pa[12] = p2h
        pa[13] = p2l
        pa[14] = ones_r
        pa[15] = ones_r
        pa_rep = np.zeros((128, R), np.float32)
        for g in range(4):
            pa_rep[32 * g : 32 * g + 16] = pa
        return pa_rep.astype(BF16)

    def pack_w(W, n_kt, n_out):
        return (
            np.asarray(W, np.float32)
            .reshape(n_kt, 128, n_out)
            .transpose(1, 0, 2)
            .reshape(128, n_kt * n_out)
            .astype(np.float16)
        )

    common = {
        "basis_aug": basis_rep,
        "w0": pack_w(W0, KT1, HID),
        "w1": pack_w(W1, KT2, HID),
        "w2": pack_w(W2, KT2, HID),
        "w3": pack_w(W3, KT2, OUT),
        "b0t": np.asarray(b0, np.float32).reshape(MT_H, 128).T.copy(),
        "b1t": np.asarray(b1, np.float32).reshape(MT_H, 128).T.copy(),
        "b2t": np.asarray(b2, np.float32).reshape(MT_H, 128).T.copy(),
        "b3t": np.asarray(b3, np.float32).reshape(MT_O, 128).T.copy(),
    }
    in_maps = []
    for c in range(NCORES):
        m = dict(common)
        m["posT_aug"] = pos_aug_for_core(c)
        in_maps.append(m)
    return in_maps


def kernel(pos, basis, W0, b0, W1, b1, W2, b2, W3, b3, _trace=False):
    if "nc" not in _cache:
        _cache["nc"] = _build_program()
    nc = _cache["nc"]
    in_maps = _prep_inputs(pos, basis, W0, b0, W1, b1, W2, b2, W3, b3)
    res = run_bass_kernel_spmd(nc, in_maps, list(range(NCORES)), trace=_trace)
    _cache["last_result"] = res
    out = np.empty((B, OUT), np.float32)
    for c in range(NCORES):
        o = np.asarray(res.results[c]["outT"])  # [MT_O, 128, BPC]
        out[c * BPC : (c + 1) * BPC] = o.transpose(2, 0, 1).reshape(BPC, OUT)
    return out


# revision 23
# speedup vs baseline: 1.0078x; 1.0008x over previous
"""Trainium2 kernel for nn_BpsMlp: KNN min-distance (B=64,N=1024 queries vs
M=4096 basis points) feeding a 4-layer MLP, data-parallel over batch across
8 NeuronCores.

Per core (8 batches = 8192 query rows):
  - distance phase: d2[q,m] accumulated exactly in fp32 PSUM via K=16
    augmented bf16 hi/lo matmuls (catastrophic-cancellation-free), four
    matmuls packed concurrently into the PE via tile_position row-groups.
  - drain: per q-tile the 4096 d2 values sit in 4 PSUM chunks of 1024.
    ScalarE (the cheaper PSUM-exit engine) casts chunks 0-2 to fp16 SBUF;
    VectorE pair-mins chunk 3 against cast2 (retiring 2 elems/cycle), folds
    the remaining fp16 with 2x-mode TTs, and finishes with a fused
    min-accumulate into the per-query x column. V-queue ordered so its
    first op per tile waits only on the first two casts.
  - x = sqrt(max(d2min, 1e-12)) with one Newton refinement step.
  - MLP in fp16 (weights streamed to SBUF during the distance phase),
    h^T layout [hid-tile 128, batch 8], relu+bias on VectorE.
"""

import sys

sys.path.insert(0, "/opt/trn_rl_repo")

import numpy as np
import ml_dtypes

import concourse.bass as bass
import concourse.mybir as mybir
import concourse.tile as tile
from concourse.bass import ds, ts
from concourse.bass_utils import run_bass_kernel_spmd

BF16 = ml_dtypes.bfloat16
DT = mybir.dt
AF = mybir.ActivationFunctionType
OP = mybir.AluOpType

B, N, M = 64, 1024, 4096
HID, OUT = 2048, 512
NCORES = 8
BPC = B // NCORES            # batches per core
R = BPC * N                  # query rows per core (8192)
QT = R // 128                # q-tiles per core (64)
KAUG = 16                    # augmented contraction dim
MT_H = HID // 128            # hid tiles (16)
KT1 = N // 128               # L1 k-tiles (8)
KT2 = HID // 128             # L2/L3/L4 k-tiles (16)
MT_O = OUT // 128            # out tiles (4)

_cache = {}


def _split_hi_lo(v):
    vh = v.astype(BF16).astype(np.float32)
    vl = (v - vh).astype(BF16).astype(np.float32)
    return vh, vl


def _build_program():
    nc = bass.Bass()

    posT = nc.declare_dram_parameter("posT_aug", [128, R], DT.bfloat16, isOutput=False)
    basisA = nc.declare_dram_parameter("basis_aug", [128, M], DT.bfloat16, isOutput=False)
    w0 = nc.declare_dram_parameter("w0", [128, KT1 * HID], DT.float16, isOutput=False)
    w1 = nc.declare_dram_parameter("w1", [128, KT2 * HID], DT.float16, isOutput=False)
    w2 = nc.declare_dram_parameter("w2", [128, KT2 * HID], DT.float16, isOutput=False)
    w3 = nc.declare_dram_parameter("w3", [128, KT2 * OUT], DT.float16, isOutput=False)
    b0d = nc.declare_dram_parameter("b0t", [128, MT_H], DT.float32, isOutput=False)
    b1d = nc.declare_dram_parameter("b1t", [128, MT_H], DT.float32, isOutput=False)
    b2d = nc.declare_dram_parameter("b2t", [128, MT_H], DT.float32, isOutput=False)
    b3d = nc.declare_dram_parameter("b3t", [128, MT_O], DT.float32, isOutput=False)
    outT = nc.declare_dram_parameter("outT", [MT_O, 128, BPC], DT.float32, isOutput=True)

    with tile.TileContext(nc) as tc:
        with (
            tc.tile_pool(name="const", bufs=1) as const,
            tc.tile_pool(name="dpsum", bufs=4, space="PSUM") as dpsum,
            tc.tile_pool(name="castp", bufs=2) as castp,
            tc.tile_pool(name="c2cp", bufs=2) as c2cp,
            tc.tile_pool(name="foldp", bufs=1) as foldp,
            tc.tile_pool(name="hpool", bufs=2) as hpool,
            tc.tile_pool(name="junkp", bufs=1) as junkp,
            tc.tile_pool(name="posc", bufs=2) as posc,
        ):
            basis_sb = const.tile([128, M], DT.bfloat16)
            pos_tiles = {}

            def issue_chunk(c, engine=None):
                # c indexes half-chunks of 512 query-columns (4 q-tiles)
                pc_ = posc.tile([128, 512], DT.bfloat16, tag="posc")
                e0 = engine if engine else nc.sync
                e0.dma_start(pc_[:], posT[:, ds(c * 512, 512)])
                pos_tiles[c] = pc_

            # ramp: spread the first DMAs across engine queues so their
            # ~0.6us issue costs don't serialize on the sync queue.
            issue_chunk(0, engine=nc.scalar)
            issue_chunk(1, engine=nc.scalar)
            # first basis quarter split in two so q-tile 0's first matmuls
            # wait on a 128KB transfer, not 256KB (both on the HWDGE sync
            # queue -- the gpsimd SWDGE path costs ~5us of Q7 descriptor
            # generation and was gating the first matmul); weight DMAs wait
            # until the second chunk boundary to keep the queues clear.
            nc.sync.dma_start(basis_sb[:, 0:512], basisA[:, 0:512])
            nc.sync.dma_start(basis_sb[:, 512:1024], basisA[:, 512:1024])
            for j in range(1, 4):
                nc.sync.dma_start(basis_sb[:, ts(j, 1024)], basisA[:, ts(j, 1024)])

            w0_sb = const.tile([128, KT1 * HID], DT.float16)
            w1_sb = const.tile([128, KT2 * HID], DT.float16)
            w2_sb = const.tile([128, KT2 * HID], DT.float16)
            w3_sb = const.tile([128, KT2 * OUT], DT.float16)
            b0_sb = const.tile([128, MT_H], DT.float32)
            b1_sb = const.tile([128, MT_H], DT.float32)
            b2_sb = const.tile([128, MT_H], DT.float32)
            b3_sb = const.tile([128, MT_O], DT.float32)

            x1 = const.tile([128, QT], DT.float32)
            junk = junkp.tile([128, 512], DT.float16)

            # MLP weight DMAs are spread across the distance phase so the
            # pos-chunk prefetches never sit behind a deep weight backlog.
            wdmas = []
            for j in range(KT1):
                wdmas.append((w0_sb[:, ts(j, HID)], w0[:, ts(j, HID)]))
            for j in range(KT2):
                wdmas.append((w1_sb[:, ts(j, HID)], w1[:, ts(j, HID)]))
                wdmas.append((w2_sb[:, ts(j, HID)], w2[:, ts(j, HID)]))
                wdmas.append((w3_sb[:, ts(j, OUT)], w3[:, ts(j, OUT)]))
            wdmas.append((b0_sb[:], b0d[:]))
            wdmas.append((b1_sb[:], b1d[:]))
            wdmas.append((b2_sb[:], b2d[:]))
            wdmas.append((b3_sb[:], b3d[:]))
            wd_i = 0

            # ---- distance phase ----
            for t in range(QT):
                if t % 4 == 0:
                    c = t // 4
                    if c + 2 < QT // 4:
                        issue_chunk(c + 2)
                    n_issue = (len(wdmas) * c) // (QT // 4 - 1) - wd_i
                    for _ in range(n_issue):
                        dst, src = wdmas[wd_i]
                        nc.sync.dma_start(dst, src)
                        wd_i += 1
                pos_chunk = pos_tiles[t // 4]
                # S-heavy drain: ScalarE casts chunks 0-2 to fp16 (its PSUM
                # exits are cheaper than VectorE's); VectorE pair-mins chunk 3
                # against cast2, then folds [cast0|cast1] + s3 with 2x fp16
                # TTs and a final fused min-accum into the x column.
                cb0 = castp.tile([128, 1024], DT.float16, tag="c01")
                cb1 = castp.tile([128, 1024], DT.float16, tag="c01")
                c2cast = c2cp.tile([128, 1024], DT.float16, tag="c2c")
                for ci in range(4):
                    pt = dpsum.tile([128, 1024], DT.float32, tag="d")
                    for h in range(2):
                        g = 2 * (ci % 2) + h
                        nc.tensor.matmul(
                            pt[:, ts(h, 512)],
                            pos_chunk[32 * g : 32 * g + KAUG, ts(t % 4, 128)],
                            basis_sb[32 * g : 32 * g + KAUG, ds(1024 * ci + 512 * h, 512)],
                            tile_position=(32 * g, 0),
                        )
                    if ci < 2:
                        nc.scalar.copy((cb0 if ci == 0 else cb1)[:], pt[:])
                        if ci == 1:
                            # issue the cast0/cast1 fold before the pair so
                            # VectorE's first op of this tile only waits on
                            # the first two casts, not the third.
                            u = foldp.tile([128, 1024], DT.float16, tag="u")
                            nc.vector.tensor_tensor(
                                u[:], cb0[:], cb1[:], op=OP.min
                            )
                    elif ci == 2:
                        nc.scalar.copy(c2cast[:], pt[:])
                    else:
                        s3 = foldp.tile([128, 1024], DT.float16, tag="s3")
                        nc.vector.tensor_tensor(s3[:], pt[:], c2cast[:], op=OP.min)
                v = foldp.tile([128, 1024], DT.float16, tag="v")
                nc.vector.tensor_tensor(v[:], u[:], s3[:], op=OP.min)
                w = foldp.tile([128, 512], DT.float16, tag="w")
                nc.vector.tensor_tensor(w[:], v[:, 0:512], v[:, 512:1024], op=OP.min)
                nc.vector.tensor_scalar(
                    junk[:], w[:], 1.0, None,
                    op0=OP.mult, op1=OP.min, accum_out=x1[:, t : t + 1],
                )

            # ---- x = sqrt(max(d2,1e-12)), one Newton step ----
            xc = const.tile([128, QT], DT.float32)
            nc.vector.tensor_scalar_max(xc[:], x1[:], 1e-12)
            y0 = const.tile([128, QT], DT.float32)
            nc.scalar.activation(y0[:], xc[:], AF.Sqrt)
            ry = const.tile([128, QT], DT.float32)
            nc.vector.reciprocal(ry[:], y0[:])
            t1 = const.tile([128, QT], DT.float32)
            nc.vector.tensor_mul(t1[:], xc[:], ry[:])
            nc.vector.tensor_add(x1[:], y0[:], t1[:])
            xbf = const.tile([128, QT], DT.float16)
            nc.vector.tensor_scalar_mul(xbf[:], x1[:], 0.5)

            # ---- MLP (h^T layout: [hid-tile 128, batch 8]) ----
            xg = xbf[:].rearrange("p (b t) -> p t b", t=KT1)
            zero_t = const.tile([128, BPC], DT.float16)
            nc.vector.memset(zero_t[:], 0.0)

            def layer(in_view, w_sb, b_sb, n_kt, n_mt, act_relu, out_dtype):
                # One small psum tile per mt-group: consecutive groups
                # ping-pong the two pool slots, so the relu's PSUM read never
                # serializes against the next group's matmuls.
                hout = hpool.tile([128, n_mt * BPC], out_dtype, tag="h" + str(n_mt))
                for mt in range(n_mt):
                    # MLP psum tiles share the distance pool's ring (first
                    # BPC columns of a [128, 1024] slot) -- saves 2 banks so
                    # the distance phase can quad-buffer.
                    ptf = dpsum.tile([128, 1024], DT.float32, tag="d")
                    pt = ptf[:, 0:BPC]
                    for kt in range(n_kt):
                        nc.tensor.matmul(
                            pt,
                            w_sb[:, ds(kt * n_mt * 128 + mt * 128, 128)],
                            in_view[:, kt, :],
                            start=(kt == 0),
                            stop=(kt == n_kt - 1),
                        )
                    if act_relu:
                        # relu(psum + bias) on VectorE (idle during MLP)
                        nc.vector.scalar_tensor_tensor(
                            hout[:, ds(mt * BPC, BPC)],
                            pt,
                            b_sb[:, mt : mt + 1],
                            zero_t[:],
                            op0=OP.add,
                            op1=OP.max,
                        )
                    else:
                        nc.scalar.activation(
                            hout[:, ds(mt * BPC, BPC)],
                            pt,
                            AF.Identity,
                            bias=b_sb[:, mt : mt + 1],
                        )
                return hout

            h1 = layer(xg, w0_sb, b0_sb, KT1, MT_H, True, DT.float16)
            h1v = h1[:].rearrange("p (t b) -> p t b", b=BPC)
            h2 = layer(h1v, w1_sb, b1_sb, KT2, MT_H, True, DT.float16)
            h2v = h2[:].rearrange("p (t b) -> p t b", b=BPC)
            h3 = layer(h2v, w2_sb, b2_sb, KT2, MT_H, True, DT.float16)
            h3v = h3[:].rearrange("p (t b) -> p t b", b=BPC)
            h4 = layer(h3v, w3_sb, b3_sb, KT2, MT_O, False, DT.float32)

            for mt in range(MT_O):
                nc.sync.dma_start(outT[mt], h4[:, ds(mt * BPC, BPC)])

    _split_multi_waits(nc)
    return nc


def _split_multi_waits(nc, max_waits=1):
    """neuronx-cc walrus rejects instructions with >1 sync wait; hoist extras
    onto nofuse NOPs just before, on the same engine."""
    ctr = 0
    for f in nc.m.functions:
        for bb in f.blocks:
            new_insts = []
            for ins in bb.instructions:
                si = getattr(ins, "sync_info", None)
                if si is not None and si.on_wait and len(si.on_wait) > max_waits:
                    waits = list(si.on_wait)
                    extra, keep = waits[:-max_waits], waits[-max_waits:]
                    for i in range(0, len(extra), max_waits):
                        ctr += 1
                        new_insts.append(
                            mybir.InstNoOp(
                                name=f"waitsplit-{ctr}",
                                engine=ins.engine,
                                sync_info=mybir.SyncInfo(
                                    on_wait=extra[i : i + max_waits], on_update=[]
                                ),
                                bass_nofuse=True,
                            )
                        )
                    si.on_wait = keep
                new_insts.append(ins)
            bb.instructions[:] = new_insts


def _prep_inputs(pos, basis, W0, b0, W1, b1, W2, b2, W3, b3):
    pos = np.asarray(pos, dtype=np.float32)
    basis = np.asarray(basis, dtype=np.float32)

    bh, bl = _split_hi_lo(basis)  # [M,3]
    q2 = (basis * basis).sum(-1)
    q2h, q2l = _split_hi_lo(q2)
    ones_m = np.ones(M, np.float32)
    basis_aug = np.zeros((16, M), np.float32)
    basis_aug[0:3] = bh.T
    basis_aug[3:6] = bh.T
    basis_aug[6:9] = bl.T
    basis_aug[9:12] = bl.T
    basis_aug[12] = ones_m
    basis_aug[13] = ones_m
    basis_aug[14] = q2h
    basis_aug[15] = q2l
    # replicate into the 4 PE row-groups (partitions 32g..32g+15)
    basis_rep = np.zeros((128, M), np.float32)
    for g in range(4):
        basis_rep[32 * g : 32 * g + 16] = basis_aug
    basis_rep = basis_rep.astype(BF16)

    def pos_aug_for_core(c):
        p = pos[c * BPC : (c + 1) * BPC].reshape(R, 3)
        a = -2.0 * p
        ah, al = _split_hi_lo(a)
        p2 = (p * p).sum(-1)
        p2h, p2l = _split_hi_lo(p2)
        ones_r = np.ones(R, np.float32)
        pa = np.zeros((16, R), np.float32)
        pa[0:3] = ah.T
        pa[3:6] = al.T
        pa[6:9] = ah.T
        pa[9:12] = al.T
        pa[12] = p2h
        pa[13] = p2l
        pa[14] = ones_r
        pa[15] = ones_r
        pa_rep = np.zeros((128, R), np.float32)
        for g in range(4):
            pa_rep[32 * g : 32 * g + 16] = pa
        return pa_rep.astype(BF16)

    def pack_w(W, n_kt, n_out):
        return (
            np.asarray(W, np.float32)
            .reshape(n_kt, 128, n_out)
            .transpose(1, 0, 2)
            .reshape(128, n_kt * n_out)
            .astype(np.float16)
        )

    common = {
        "basis_aug": basis_rep,
        "w0": pack_w(W0, KT1, HID),
        "w1": pack_w(W1, KT2, HID),
        "w2": pack_w(W2, KT2, HID),
        "w3": pack_w(W3, KT2, OUT),
        "b0t": np.asarray(b0, np.float32).reshape(MT_H, 128).T.copy(),
        "b1t": np.asarray(b1, np.float32).reshape(MT_H, 128).T.copy(),
        "b2t": np.asarray(b2, np.float32).reshape(MT_H, 128).T.copy(),
        "b3t": np.asarray(b3, np.float32).reshape(MT_O, 128).T.copy(),
    }
    in_maps = []
    for c in range(NCORES):
        m = dict(common)
        m["posT_aug"] = pos_aug_for_core(c)
        in_maps.append(m)
    return in_maps


def kernel(pos, basis, W0, b0, W1, b1, W2, b2, W3, b3, _trace=False):
    if "nc" not in _cache:
        _cache["nc"] = _build_program()
    nc = _cache["nc"]
    in_maps = _prep_inputs(pos, basis, W0, b0, W1, b1, W2, b2, W3, b3)
    res = run_bass_kernel_spmd(nc, in_maps, list(range(NCORES)), trace=_trace)
    _cache["last_result"] = res
    out = np.empty((B, OUT), np.float32)
    for c in range(NCORES):
        o = np.asarray(res.results[c]["outT"])  # [MT_O, 128, BPC]
        out[c * BPC : (c + 1) * BPC] = o.transpose(2, 0, 1).reshape(BPC, OUT)
    return out


# revision 24
# speedup vs baseline: 1.0257x; 1.0177x over previous
"""Trainium2 kernel for nn_BpsMlp: KNN min-distance (B=64,N=1024 queries vs
M=4096 basis points) feeding a 4-layer MLP, data-parallel over batch across
8 NeuronCores.

Per core (8 batches = 8192 query rows):
  - distance phase: d2[q,m] accumulated exactly in fp32 PSUM via K=16
    augmented bf16 hi/lo matmuls (catastrophic-cancellation-free), four
    matmuls packed concurrently into the PE via tile_position row-groups.
  - drain: per q-tile the 4096 d2 values sit in 4 PSUM chunks of 1024.
    ScalarE (the cheaper PSUM-exit engine) casts chunks 0-2 to fp16 SBUF;
    VectorE pair-mins chunk 3 against cast2 (retiring 2 elems/cycle), folds
    the remaining fp16 with 2x-mode TTs, and finishes with a fused
    min-accumulate into the per-query x column. V-queue ordered so its
    first op per tile waits only on the first two casts.
  - x = sqrt(max(d2min, 1e-12)) with one Newton refinement step.
  - MLP in fp16 (weights streamed to SBUF during the distance phase),
    h^T layout [hid-tile 128, batch 8], relu+bias on VectorE.
"""

import sys

sys.path.insert(0, "/opt/trn_rl_repo")

import numpy as np
import ml_dtypes

import concourse.bass as bass
import concourse.mybir as mybir
import concourse.tile as tile
from concourse.bass import ds, ts
from concourse.bass_utils import run_bass_kernel_spmd

BF16 = ml_dtypes.bfloat16
DT = mybir.dt
AF = mybir.ActivationFunctionType
OP = mybir.AluOpType

B, N, M = 64, 1024, 4096
HID, OUT = 2048, 512
NCORES = 8
BPC = B // NCORES            # batches per core
R = BPC * N                  # query rows per core (8192)
QT = R // 128                # q-tiles per core (64)
KAUG = 16                    # augmented contraction dim
MT_H = HID // 128            # hid tiles (16)
KT1 = N // 128               # L1 k-tiles (8)
KT2 = HID // 128             # L2/L3/L4 k-tiles (16)
MT_O = OUT // 128            # out tiles (4)

_cache = {}


def _split_hi_lo(v):
    vh = v.astype(BF16).astype(np.float32)
    vl = (v - vh).astype(BF16).astype(np.float32)
    return vh, vl


def _build_program():
    nc = bass.Bass()

    posT = nc.declare_dram_parameter("posT_aug", [128, R], DT.bfloat16, isOutput=False)
    basisA = nc.declare_dram_parameter("basis_aug", [128, M], DT.bfloat16, isOutput=False)
    w0 = nc.declare_dram_parameter("w0", [128, KT1 * HID], DT.float16, isOutput=False)
    w1 = nc.declare_dram_parameter("w1", [128, KT2 * HID], DT.float16, isOutput=False)
    w2 = nc.declare_dram_parameter("w2", [128, KT2 * HID], DT.float16, isOutput=False)
    w3 = nc.declare_dram_parameter("w3", [128, KT2 * OUT], DT.float16, isOutput=False)
    b0d = nc.declare_dram_parameter("b0t", [128, MT_H], DT.float32, isOutput=False)
    b1d = nc.declare_dram_parameter("b1t", [128, MT_H], DT.float32, isOutput=False)
    b2d = nc.declare_dram_parameter("b2t", [128, MT_H], DT.float32, isOutput=False)
    b3d = nc.declare_dram_parameter("b3t", [128, MT_O], DT.float32, isOutput=False)
    outT = nc.declare_dram_parameter("outT", [MT_O, 128, BPC], DT.float32, isOutput=True)

    with tile.TileContext(nc) as tc:
        with (
            tc.tile_pool(name="const", bufs=1) as const,
            tc.tile_pool(name="dpsum", bufs=4, space="PSUM") as dpsum,
            tc.tile_pool(name="castp", bufs=2) as castp,
            tc.tile_pool(name="c2cp", bufs=2) as c2cp,
            tc.tile_pool(name="foldp", bufs=1) as foldp,
            tc.tile_pool(name="hpool", bufs=2) as hpool,
            tc.tile_pool(name="junkp", bufs=1) as junkp,
            tc.tile_pool(name="posc", bufs=2) as posc,
        ):
            basis_sb = const.tile([128, M], DT.bfloat16)
            pos_tiles = {}

            def issue_chunk(c, engine=None):
                # c indexes half-chunks of 512 query-columns (4 q-tiles)
                pc_ = posc.tile([128, 512], DT.bfloat16, tag="posc")
                e0 = engine if engine else nc.sync
                e0.dma_start(pc_[:], posT[:, ds(c * 512, 512)])
                pos_tiles[c] = pc_

            # ramp: spread the first DMAs across engine queues so their
            # ~0.6us issue costs don't serialize on the sync queue.
            issue_chunk(0, engine=nc.scalar)
            issue_chunk(1, engine=nc.scalar)
            # first basis quarter split in two so q-tile 0's first matmuls
            # wait on a 128KB transfer, not 256KB (both on the HWDGE sync
            # queue -- the gpsimd SWDGE path costs ~5us of Q7 descriptor
            # generation and was gating the first matmul); weight DMAs wait
            # until the second chunk boundary to keep the queues clear.
            nc.sync.dma_start(basis_sb[:, 0:512], basisA[:, 0:512])
            nc.sync.dma_start(basis_sb[:, 512:1024], basisA[:, 512:1024])
            for j in range(1, 4):
                nc.sync.dma_start(basis_sb[:, ts(j, 1024)], basisA[:, ts(j, 1024)])

            w0_sb = const.tile([128, KT1 * HID], DT.float16)
            w1_sb = const.tile([128, KT2 * HID], DT.float16)
            w2_sb = const.tile([128, KT2 * HID], DT.float16)
            w3_sb = const.tile([128, KT2 * OUT], DT.float16)
            b0_sb = const.tile([128, MT_H], DT.float32)
            b1_sb = const.tile([128, MT_H], DT.float32)
            b2_sb = const.tile([128, MT_H], DT.float32)
            b3_sb = const.tile([128, MT_O], DT.float32)

            x1 = const.tile([128, QT], DT.float32)
            junk = junkp.tile([128, 512], DT.float16)
            # v/w fold buffers hold TWO q-tiles side by side: the 512-wide
            # halving fold runs once per tile-pair as a single 3D-AP TT
            # (pages = the two tiles), amortizing the DVE fixed cost.
            vbuf = const.tile([128, 2048], DT.float16)
            wbuf = const.tile([128, 1024], DT.float16)

            # MLP weight DMAs are spread across the distance phase so the
            # pos-chunk prefetches never sit behind a deep weight backlog.
            wdmas = []
            for j in range(KT1):
                wdmas.append((w0_sb[:, ts(j, HID)], w0[:, ts(j, HID)]))
            for j in range(KT2):
                wdmas.append((w1_sb[:, ts(j, HID)], w1[:, ts(j, HID)]))
                wdmas.append((w2_sb[:, ts(j, HID)], w2[:, ts(j, HID)]))
                wdmas.append((w3_sb[:, ts(j, OUT)], w3[:, ts(j, OUT)]))
            wdmas.append((b0_sb[:], b0d[:]))
            wdmas.append((b1_sb[:], b1d[:]))
            wdmas.append((b2_sb[:], b2d[:]))
            wdmas.append((b3_sb[:], b3d[:]))
            wd_i = 0

            # ---- distance phase ----
            for t in range(QT):
                if t % 4 == 0:
                    c = t // 4
                    if c + 2 < QT // 4:
                        issue_chunk(c + 2)
                    n_issue = (len(wdmas) * c) // (QT // 4 - 1) - wd_i
                    for _ in range(n_issue):
                        dst, src = wdmas[wd_i]
                        nc.sync.dma_start(dst, src)
                        wd_i += 1
                pos_chunk = pos_tiles[t // 4]
                # S-heavy drain: ScalarE casts chunks 0-2 to fp16 (its PSUM
                # exits are cheaper than VectorE's); VectorE pair-mins chunk 3
                # against cast2, then folds [cast0|cast1] + s3 with 2x fp16
                # TTs and a final fused min-accum into the x column.
                cb0 = castp.tile([128, 1024], DT.float16, tag="c01")
                cb1 = castp.tile([128, 1024], DT.float16, tag="c01")
                c2cast = c2cp.tile([128, 1024], DT.float16, tag="c2c")
                for ci in range(4):
                    pt = dpsum.tile([128, 1024], DT.float32, tag="d")
                    for h in range(2):
                        g = 2 * (ci % 2) + h
                        nc.tensor.matmul(
                            pt[:, ts(h, 512)],
                            pos_chunk[32 * g : 32 * g + KAUG, ts(t % 4, 128)],
                            basis_sb[32 * g : 32 * g + KAUG, ds(1024 * ci + 512 * h, 512)],
                            tile_position=(32 * g, 0),
                        )
                    if ci < 2:
                        nc.scalar.copy((cb0 if ci == 0 else cb1)[:], pt[:])
                        if ci == 1:
                            # issue the cast0/cast1 fold before the pair so
                            # VectorE's first op of this tile only waits on
                            # the first two casts, not the third.
                            u = foldp.tile([128, 1024], DT.float16, tag="u")
                            nc.vector.tensor_tensor(
                                u[:], cb0[:], cb1[:], op=OP.min
                            )
                    elif ci == 2:
                        nc.scalar.copy(c2cast[:], pt[:])
                    else:
                        s3 = foldp.tile([128, 1024], DT.float16, tag="s3")
                        nc.vector.tensor_tensor(s3[:], pt[:], c2cast[:], op=OP.min)
                half = t % 2
                nc.vector.tensor_tensor(
                    vbuf[:, ds(half * 1024, 1024)], u[:], s3[:], op=OP.min
                )
                if half == 1:
                    # one batched halving fold + the two accumulates; all on
                    # VectorE, so plain program order keeps it correct.
                    v3 = vbuf[:].rearrange("p (s n) -> p s n", s=2)
                    w3 = wbuf[:].rearrange("p (s n) -> p s n", s=2)
                    nc.vector.tensor_tensor(
                        w3, v3[:, :, 0:512], v3[:, :, 512:1024], op=OP.min
                    )
                    nc.vector.tensor_scalar(
                        junk[:], wbuf[:, 0:512], 1.0, None,
                        op0=OP.mult, op1=OP.min, accum_out=x1[:, t - 1 : t],
                    )
                    nc.vector.tensor_scalar(
                        junk[:], wbuf[:, 512:1024], 1.0, None,
                        op0=OP.mult, op1=OP.min, accum_out=x1[:, t : t + 1],
                    )

            # ---- x = sqrt(max(d2,1e-12)), one Newton step ----
            xc = const.tile([128, QT], DT.float32)
            nc.vector.tensor_scalar_max(xc[:], x1[:], 1e-12)
            y0 = const.tile([128, QT], DT.float32)
            nc.scalar.activation(y0[:], xc[:], AF.Sqrt)
            ry = const.tile([128, QT], DT.float32)
            nc.vector.reciprocal(ry[:], y0[:])
            t1 = const.tile([128, QT], DT.float32)
            nc.vector.tensor_mul(t1[:], xc[:], ry[:])
            nc.vector.tensor_add(x1[:], y0[:], t1[:])
            xbf = const.tile([128, QT], DT.float16)
            nc.vector.tensor_scalar_mul(xbf[:], x1[:], 0.5)

            # ---- MLP (h^T layout: [hid-tile 128, batch 8]) ----
            xg = xbf[:].rearrange("p (b t) -> p t b", t=KT1)
            zero_t = const.tile([128, BPC], DT.float16)
            nc.vector.memset(zero_t[:], 0.0)

            def layer(in_view, w_sb, b_sb, n_kt, n_mt, act_relu, out_dtype):
                # One small psum tile per mt-group: consecutive groups
                # ping-pong the two pool slots, so the relu's PSUM read never
                # serializes against the next group's matmuls.
                hout = hpool.tile([128, n_mt * BPC], out_dtype, tag="h" + str(n_mt))
                for mt in range(n_mt):
                    # MLP psum tiles share the distance pool's ring (first
                    # BPC columns of a [128, 1024] slot) -- saves 2 banks so
                    # the distance phase can quad-buffer.
                    ptf = dpsum.tile([128, 1024], DT.float32, tag="d")
                    pt = ptf[:, 0:BPC]
                    for kt in range(n_kt):
                        nc.tensor.matmul(
                            pt,
                            w_sb[:, ds(kt * n_mt * 128 + mt * 128, 128)],
                            in_view[:, kt, :],
                            start=(kt == 0),
                            stop=(kt == n_kt - 1),
                        )
                    if act_relu:
                        # relu(psum + bias) on VectorE (idle during MLP)
                        nc.vector.scalar_tensor_tensor(
                            hout[:, ds(mt * BPC, BPC)],
                            pt,
                            b_sb[:, mt : mt + 1],
                            zero_t[:],
                            op0=OP.add,
                            op1=OP.max,
                        )
                    else:
                        nc.scalar.activation(
                            hout[:, ds(mt * BPC, BPC)],
                            pt,
                            AF.Identity,
                            bias=b_sb[:, mt : mt + 1],
                        )
                return hout

            h1 = layer(xg, w0_sb, b0_sb, KT1, MT_H, True, DT.float16)
            h1v = h1[:].rearrange("p (t b) -> p t b", b=BPC)
            h2 = layer(h1v, w1_sb, b1_sb, KT2, MT_H, True, DT.float16)
            h2v = h2[:].rearrange("p (t b) -> p t b", b=BPC)
            h3 = layer(h2v, w2_sb, b2_sb, KT2, MT_H, True, DT.float16)
            h3v = h3[:].rearrange("p (t b) -> p t b", b=BPC)
            h4 = layer(h3v, w3_sb, b3_sb, KT2, MT_O, False, DT.float32)

            for mt in range(MT_O):
                nc.sync.dma_start(outT[mt], h4[:, ds(mt * BPC, BPC)])

    _split_multi_waits(nc)
    return nc


def _split_multi_waits(nc, max_waits=1):
    """neuronx-cc walrus rejects instructions with >1 sync wait; hoist extras
    onto nofuse NOPs just before, on the same engine."""
    ctr = 0
    for f in nc.m.functions:
        for bb in f.blocks:
            new_insts = []
            for ins in bb.instructions:
                si = getattr(ins, "sync_info", None)
                if si is not None and si.on_wait and len(si.on_wait) > max_waits:
                    waits = list(si.on_wait)
                    extra, keep = waits[:-max_waits], waits[-max_waits:]
                    for i in range(0, len(extra), max_waits):
                        ctr += 1
                        new_insts.append(
                            mybir.InstNoOp(
                                name=f"waitsplit-{ctr}",
                                engine=ins.engine,
                                sync_info=mybir.SyncInfo(
                                    on_wait=extra[i : i + max_waits], on_update=[]
                                ),
                                bass_nofuse=True,
                            )
                        )
                    si.on_wait = keep
                new_insts.append(ins)
            bb.instructions[:] = new_insts


def _prep_inputs(pos, basis, W0, b0, W1, b1, W2, b2, W3, b3):
    pos = np.asarray(pos, dtype=np.float32)
    basis = np.asarray(basis, dtype=np.float32)

    bh, bl = _split_hi_lo(basis)  # [M,3]
    q2 = (basis * basis).sum(-1)
    q2h, q2l = _split_hi_lo(q2)
    ones_m = np.ones(M, np.float32)
    basis_aug = np.zeros((16, M), np.float32)
    basis_aug[0:3] = bh.T
    basis_aug[3:6] = bh.T
    basis_aug[6:9] = bl.T
    basis_aug[9:12] = bl.T
    basis_aug[12] = ones_m
    basis_aug[13] = ones_m
    basis_aug[14] = q2h
    basis_aug[15] = q2l
    # replicate into the 4 PE row-groups (partitions 32g..32g+15)
    basis_rep = np.zeros((128, M), np.float32)
    for g in range(4):
        basis_rep[32 * g : 32 * g + 16] = basis_aug
    basis_rep = basis_rep.astype(BF16)

    def pos_aug_for_core(c):
        p = pos[c * BPC : (c + 1) * BPC].reshape(R, 3)
        a = -2.0 * p
        ah, al = _split_hi_lo(a)
        p2 = (p * p).sum(-1)
        p2h, p2l = _split_hi_lo(p2)
        ones_r = np.ones(R, np.float32)
        pa = np.zeros((16, R), np.float32)
        pa[0:3] = ah.T
        pa[3:6] = al.T
        pa[6:9] = ah.T
        pa[9:12] = al.T
        pa[12] = p2h
        pa[13] = p2l
        pa[14] = ones_r
        pa[15] = ones_r
        pa_rep = np.zeros((128, R), np.float32)
        for g in range(4):
            pa_rep[32 * g : 32 * g + 16] = pa
        return pa_rep.astype(BF16)

    def pack_w(W, n_kt, n_out):
        return (
            np.asarray(W, np.float32)
            .reshape(n_kt, 128, n_out)
            .transpose(1, 0, 2)
            .reshape(128, n_kt * n_out)
            .astype(np.float16)
        )

    common = {
        "basis_aug": basis_rep,
        "w0": pack_w(W0, KT1, HID),
        "w1": pack_w(W1, KT2, HID),
        "w2": pack_w(W2, KT2, HID),
        "w3": pack_w(W3, KT2, OUT),
        "b0t": np.asarray(b0, np.float32).reshape(MT_H, 128).T.copy(),
        "b1t": np.asarray(b1, np.float32).reshape(MT_H, 128).T.copy(),
        "b2t": np.asarray(b2, np.float32).reshape(MT_H, 128).T.copy(),
        "b3t": np.asarray(b3, np.float32).reshape(MT_O, 128).T.copy(),
    }
    in_maps = []
    for c in range(NCORES):
        m = dict(common)
        m["posT_aug"] = pos_aug_for_core(c)
        in_maps.append(m)
    return in_maps


def kernel(pos, basis, W0, b0, W1, b1, W2, b2, W3, b3, _trace=False):
    if "nc" not in _cache:
        _cache["nc"] = _build_program()
    nc = _cache["nc"]
    in_maps = _prep_inputs(pos, basis, W0, b0, W1, b1, W2, b2, W3, b3)
    res = run_bass_kernel_spmd(nc, in_maps, list(range(NCORES)), trace=_trace)
    _cache["last_result"] = res
    out = np.empty((B, OUT), np.float32)
    for c in range(NCORES):
        o = np.asarray(res.results[c]["outT"])  # [MT_O, 128, BPC]
        out[c * BPC : (c + 1) * BPC] = o.transpose(2, 0, 1).reshape(BPC, OUT)
    return out
